# revision 17
# baseline (speedup 1.0000x reference)
"""CrossAttnBlock on 8 trn2 NeuronCores, v2.

Sharding: core c -> batch b=c//4, head-quad hq=c%4 (4 of 16 heads).
Attention is Megatron-sliced over heads; the out-projection partial sums
are combined with TWO half-sized bf16 ReduceScatters so the first one
overlaps the second half of attention.  Host folds LN affine params into
the projection weights (exact), combines the two bias mats with their
softplus()'d scalar gates, and pre-transposes that combined bias to
[S, K] so the device needs no bias transposes at all.  K tokens are fed
in a block-permuted order so each half-RS delivers exactly the rows this
core's FFN tail needs.  exp(bias*gate) (the "ET" tensor) is computed
just-in-time inside the first attention half and its second-K-half is
cached for the second attention half.
"""
import sys
import numpy as np

sys.path.insert(0, "/opt/trn_rl_repo")

import ml_dtypes  # noqa: E402
import concourse.bass as bass  # noqa: E402
import concourse.mybir as mybir  # noqa: E402
import concourse.tile as tile  # noqa: E402
from concourse import bacc  # noqa: E402
from concourse import bass_utils  # noqa: E402

F32 = mybir.dt.float32
BF16 = mybir.dt.bfloat16
AF = mybir.ActivationFunctionType
OP = mybir.AluOpType

D = 1024
H = 16
HD = 64
B = 2
K = 1024
S = 4096
EPS = 1e-5
N_CORES = 8
KQ = K // 4          # rows per core owned by the FFN tail
KH = K // 2          # attention K-half
HC = 4               # heads per core
DH = HC * HD         # ctx dims per core (256)
P = 128
DC = D // P          # 8 D-chunks
D2 = 2 * D
RING = 8             # ET ring slots (first-half columns)
PRIME = 6            # ET chunks computed before attention starts

_CACHE = {}


def _ln_stats(nc, pool, xt, n_free):
    """bn_stats/aggr -> (neg_mean*rs, rs) [P,1] f32 tiles (rs=1/std)."""
    n_sub = (n_free + 511) // 512
    st = pool.tile([P, n_sub, 6], F32, tag="ln_st")
    xs = xt.rearrange("p (s f) -> p s f", s=n_sub)
    for i in range(n_sub):
        nc.vector.bn_stats(out=st[:, i, :], in_=xs[:, i, :])
    mv = pool.tile([P, 2], F32, tag="ln_mv")
    nc.vector.bn_aggr(out=mv, in_=st[:, :, :])
    rs = pool.tile([P, 1], F32, tag="ln_rs")
    nc.scalar.activation(rs, mv[:, 1:2], AF.Sqrt, bias=nc._eps_t[:, :], scale=1.0)
    nc.vector.reciprocal(rs, rs)
    return mv, rs


def _build_nc(taps=False):
    nc = bacc.Bacc("TRN2", target_bir_lowering=False, debug=False,
                   num_devices=N_CORES)

    dt_in = {}
    def din(name, shape, dt=F32):
        dt_in[name] = nc.dram_tensor(name, shape, dt, kind="ExternalInput")
        return dt_in[name]

    q_d = din("q", [K, D], BF16)          # permuted tokens
    kv_d = din("kv", [S, D], BF16)
    bt_d = din("bt", [S, K])              # (c1*ab+c2*ob)^T, cols permuted
    gate_d = din("gate", [K, 1])          # permuted
    gbc_d = din("gbc", [P, K])            # gate broadcast to 128 rows
    qres_d = din("q_res", [KQ, D])        # original row order
    wq_d = din("wq", [D, DH], BF16)
    wk_d = din("wk", [D, DH], BF16)
    wv_d = din("wv", [D, DH], BF16)
    bq_d = din("bq", [1, DH], BF16)
    bk_d = din("bk", [1, DH], BF16)
    bv_d = din("bv", [1, DH], BF16)
    grow_d = din("growb", [1, K], BF16)   # gate row (permuted)
    wo_d = din("wo", [DH, D], BF16)
    bo_d = din("bo", [1, D])
    w1_d = din("w1", [D, D2], BF16)
    b1_d = din("b1", [1, D2], BF16)
    w2_d = din("w2", [D2, D], BF16)
    b2_d = din("b2", [1, D], BF16)
    out_d = nc.dram_tensor("xq", [KQ, D], F32, kind="ExternalOutput")

    rs_out = [nc.dram_tensor(f"rs_out{i}", [KQ // 2, D], BF16)
              for i in range(2)]
    tap = {}
    if taps:
        tap["qpT"] = nc.dram_tensor("t_qpT", [P, 2, K], BF16, kind="ExternalOutput")
        tap["kpT"] = nc.dram_tensor("t_kpT", [P, 2, S], BF16, kind="ExternalOutput")
        tap["vp"] = nc.dram_tensor("t_vp", [P, 32, HC * 65], BF16, kind="ExternalOutput")
        tap["etB"] = nc.dram_tensor("t_etB", [P, 32, KH], BF16, kind="ExternalOutput")
        tap["ctx"] = nc.dram_tensor("t_ctx", [P, 2, K], BF16, kind="ExternalOutput")
        tap["y"] = nc.dram_tensor("t_y", [P, 4, D], BF16, kind="ExternalOutput")
        tap["x"] = nc.dram_tensor("t_x", [P, 2, D], F32, kind="ExternalOutput")

    groups = [[0, 1, 2, 3], [4, 5, 6, 7]]

    with tile.TileContext(nc) as tc:
        with (
            tc.tile_pool(name="const", bufs=1) as cpool,
            tc.tile_pool(name="persist", bufs=1) as pp,
            tc.tile_pool(name="dram", bufs=1, space="DRAM") as dpool,
        ):
            # ---- constants ----
            eps_t = cpool.tile([P, 1], F32)
            nc.vector.memset(eps_t, EPS)
            nc._eps_t = eps_t
            ones_row = cpool.tile([1, 512], BF16)
            nc.vector.memset(ones_row, 1.0)
            ones64 = cpool.tile([1, 64], F32)
            nc.vector.memset(ones64, 1.0)
            gsb = cpool.tile([P, DC], F32)
            nc.sync.dma_start(out=gsb, in_=gate_d.ap().rearrange(
                "(t p) o -> p (t o)", p=P))
            gbc = cpool.tile([P, K], F32)
            nc.sync.dma_start(out=gbc, in_=gbc_d[:, :])
            grow_bf = cpool.tile([1, K], BF16)
            nc.sync.dma_start(out=grow_bf, in_=grow_d[:, :])
            bq_bf = cpool.tile([1, DH], BF16)
            nc.sync.dma_start(out=bq_bf, in_=bq_d[:, :])
            bk_bf = cpool.tile([1, DH], BF16)
            nc.sync.dma_start(out=bk_bf, in_=bk_d[:, :])
            bv_bf = cpool.tile([1, DH], BF16)
            nc.sync.dma_start(out=bv_bf, in_=bv_d[:, :])
            b1_bf = cpool.tile([1, D2], BF16)
            nc.sync.dma_start(out=b1_bf, in_=b1_d[:, :])
            b2_bf = cpool.tile([1, D], BF16)
            nc.sync.dma_start(out=b2_bf, in_=b2_d[:, :])

            # ---- persistent activation tensors ----
            qpT = pp.tile([P, 2, K], BF16)       # [2 heads x 64, hp, k]
            kpT = pp.tile([P, 2, S], BF16)
            vp = pp.tile([P, 32, HC * 65], BF16)  # [s%128, s//128, h*65+(hd|one)]
            ctxT = pp.tile([P, 2, K], BF16)
            qres_sb = pp.tile([P, 2, D], F32)
            nc.sync.dma_start(out=qres_sb, in_=qres_d.ap().rearrange(
                "(t p) d -> p t d", p=P))
            bo_bc = pp.tile([P, D], F32)
            nc.sync.dma_start(out=bo_bc, in_=bass.AP(
                tensor=bo_d, offset=0, ap=[[0, P], [1, D]]))

            # ---- weights (all early; DMA engines fill during front) ----
            wq_bf = pp.tile([P, DC, DH], BF16)
            nc.scalar.dma_start(out=wq_bf, in_=wq_d.ap().rearrange(
                "(c p) n -> p c n", p=P))
            wk_bf = pp.tile([P, DC, DH], BF16)
            nc.scalar.dma_start(out=wk_bf, in_=wk_d.ap().rearrange(
                "(c p) n -> p c n", p=P))
            wv_bf = pp.tile([P, DC, DH], BF16)
            nc.scalar.dma_start(out=wv_bf, in_=wv_d.ap().rearrange(
                "(c p) n -> p c n", p=P))
            wo_bf = pp.tile([P, 2, D], BF16)
            nc.scalar.dma_start(out=wo_bf, in_=wo_d.ap().rearrange(
                "(c p) n -> p c n", p=P))
            w1_bf = pp.tile([P, DC, D2], BF16)
            nc.scalar.dma_start(out=w1_bf, in_=w1_d.ap().rearrange(
                "(c p) n -> p c n", p=P))

            # ones columns of vp
            for h in range(HC):
                nc.vector.memset(vp[:, :, h * 65 + 64:h * 65 + 65], 1.0)

            # ============ q: LN(+gate fold) -> transpose -> projection ====
            # LN apply on the scalar engine: out = Identity(x*rsg + (-mu*rsg))
            with (
                tc.tile_pool(name="psA", bufs=4, space="PSUM") as psA,
                tc.tile_pool(name="lnst", bufs=5) as lpool,
                tc.tile_pool(name="xT", bufs=3) as xTp,
            ):
                for blk in range(K // 512):
                    qT = xTp.tile([P, DC, 512], BF16, tag="xT")
                    for tt in range(4):
                        t = blk * 4 + tt
                        qt = lpool.tile([P, D], BF16, tag="ln_in")
                        nc.sync.dma_start(out=qt, in_=q_d[t * P:(t + 1) * P, :])
                        mv, rs = _ln_stats(nc, lpool, qt, D)
                        rsg = lpool.tile([P, 1], F32, tag="rsg")
                        nc.vector.tensor_tensor(out=rsg, in0=rs,
                                                in1=gsb[:, t:t + 1], op=OP.mult)
                        qn = lpool.tile([P, D], BF16, tag="ln_out")
                        nc.vector.tensor_scalar(out=qn, in0=qt,
                                                scalar1=mv[:, 0:1], scalar2=rsg,
                                                op0=OP.subtract, op1=OP.mult)
                        nc.scalar.dma_start_transpose(
                            qT[:, :, tt * P:(tt + 1) * P], qn[:, :])
                    for hp in range(2):
                        ps = psA.tile([P, 512], F32, tag="mm")
                        for dc in range(DC):
                            nc.tensor.matmul(
                                ps[:, :],
                                wq_bf[:, dc, hp * P:(hp + 1) * P],
                                qT[:, dc, :],
                                start=(dc == 0), stop=False)
                        nc.tensor.matmul(
                            ps[:, :], bq_bf[0:1, hp * P:(hp + 1) * P],
                            grow_bf[0:1, blk * 512:(blk + 1) * 512],
                            start=False, stop=True)
                        nc.vector.tensor_copy(
                            qpT[:, hp, blk * 512:(blk + 1) * 512], ps[:, :])

                # ============ kv: LN -> transpose -> k/v projections =====
                for blk in range(S // 512):
                    kvT = xTp.tile([P, DC, 512], BF16, tag="xT")
                    for tt in range(4):
                        t = blk * 4 + tt
                        xt = lpool.tile([P, D], BF16, tag="ln_in")
                        nc.sync.dma_start(out=xt, in_=kv_d[t * P:(t + 1) * P, :])
                        mv, rs = _ln_stats(nc, lpool, xt, D)
                        xn = lpool.tile([P, D], BF16, tag="ln_out")
                        nc.vector.tensor_scalar(out=xn, in0=xt,
                                                scalar1=mv[:, 0:1], scalar2=rs,
                                                op0=OP.subtract, op1=OP.mult)
                        nc.scalar.dma_start_transpose(
                            kvT[:, :, tt * P:(tt + 1) * P], xn[:, :])
                    sl = slice(blk * 512, (blk + 1) * 512)
                    for hp in range(2):
                        ps = psA.tile([P, 512], F32, tag="mm")
                        for dc in range(DC):
                            nc.tensor.matmul(
                                ps[:, :], wk_bf[:, dc, hp * P:(hp + 1) * P],
                                kvT[:, dc, :], start=(dc == 0), stop=False)
                        nc.tensor.matmul(
                            ps[:, :], bk_bf[0:1, hp * P:(hp + 1) * P],
                            ones_row[0:1, :], start=False, stop=True)
                        nc.vector.tensor_copy(kpT[:, hp, sl], ps[:, :])
                    # v in [s-part, head*64] layout directly: kvT chunk is
                    # the stationary operand, wv the moving one.
                    for sch in range(4):
                        sc = blk * 4 + sch
                        ps = psA.tile([P, DH], F32, tag="vmm")
                        for dc in range(DC):
                            nc.tensor.matmul(
                                ps[:, :],
                                kvT[:, dc, sch * P:(sch + 1) * P],
                                wv_bf[:, dc, :], start=(dc == 0), stop=False)
                        nc.tensor.matmul(
                            ps[:, :], ones_row[0:1, 0:P], bv_bf[0:1, :],
                            start=False, stop=True)
                        for h in range(HC):
                            nc.vector.tensor_copy(
                                vp[:, sc, h * 65:h * 65 + 64],
                                ps[:, h * 64:(h + 1) * 64])

            if taps:
                nc.sync.dma_start(out=tap["qpT"].ap(), in_=qpT[:, :, :])
                nc.sync.dma_start(out=tap["kpT"].ap(), in_=kpT[:, :, :])
                nc.sync.dma_start(out=tap["vp"].ap(), in_=vp[:, :, :])

            # ======================= attention ==========================
            # ET chunk j (s rows j*128..j*128+127):
            #   cb = bt_chunk * gbc ; etA_ring[j%RING] = exp(cb[:, :KH])
            #   etB[:, j, :] = exp(cb[:, KH:])
            et_cm = tc.tile_pool(name="etB", bufs=1)
            etp = et_cm.__enter__()
            etB = etp.tile([P, 32, KH], BF16)
            ring_cm = tc.tile_pool(name="etring", bufs=1)
            rgp = ring_cm.__enter__()
            etA = rgp.tile([P, RING, KH], BF16)

            def emit_et(j, btp, cbp):
                btt = btp.tile([P, K], F32, tag="bt")
                nc.gpsimd.dma_start(out=btt, in_=bt_d[j * P:(j + 1) * P, :])
                cb = cbp.tile([P, K], F32, tag="cb")
                nc.vector.tensor_tensor(out=cb, in0=btt, in1=gbc, op=OP.mult)
                nc.scalar.activation(etA[:, j % RING, :], cb[:, 0:KH], AF.Exp)
                nc.scalar.activation(etB[:, j, :], cb[:, KH:K], AF.Exp)

            with (
                tc.tile_pool(name="btst", bufs=2) as btp,
                tc.tile_pool(name="cbst", bufs=2) as cbp,
                tc.tile_pool(name="att", bufs=8) as apool,
                tc.tile_pool(name="eqp", bufs=3) as eqpool,
                tc.tile_pool(name="rrp", bufs=2) as rrpool,
                tc.tile_pool(name="yst", bufs=1) as ypool,
            ):
                for j in range(PRIME):
                    emit_et(j, btp, cbp)
                for half in range(2):
                    ksl = slice(half * KH, (half + 1) * KH)
                    with tc.tile_pool(name="psPV", bufs=1, space="PSUM") as psPV:
                        pvs = [psPV.tile([65, 512], F32, tag=f"pv{h}",
                                         name=f"pv_{half}_{h}")
                               for h in range(HC)]
                        with tc.tile_pool(name="psS", bufs=2,
                                          space="PSUM") as psS:
                            prev = None
                            for sc in range(S // P):
                                if half == 0 and sc + PRIME < 32:
                                    emit_et(sc + PRIME, btp, cbp)
                                cur = []
                                pair_ps = []
                                for pr in range(2):
                                    sps = psS.tile([P, 1024], F32, tag="sc",
                                                   name=f"sps{half}_{sc}_{pr}")
                                    for i in range(2):
                                        h = pr * 2 + i
                                        pb = (h % 2) * 64
                                        hp = h // 2
                                        nc.tensor.matmul(
                                            sps[:, i * 512:(i + 1) * 512],
                                            kpT[pb:pb + 64, hp,
                                                sc * P:(sc + 1) * P],
                                            qpT[pb:pb + 64, hp, ksl],
                                            start=True, stop=True)
                                    pair_ps.append(sps)
                                src = (etA[:, sc % RING, :] if half == 0
                                       else etB[:, sc, :])
                                for pr in range(2):
                                    eq = eqpool.tile([P, 1024], BF16, tag="eq")
                                    nc.scalar.activation(eq, pair_ps[pr][:, :],
                                                         AF.Exp)
                                    for i in range(2):
                                        h = pr * 2 + i
                                        at = apool.tile([P, 512], BF16,
                                                        tag="at")
                                        nc.vector.tensor_tensor(
                                            out=at,
                                            in0=eq[:, i * 512:(i + 1) * 512],
                                            in1=src, op=OP.mult)
                                        cur.append((h, at))
                                if prev is not None:
                                    for h, at in prev:
                                        nc.tensor.matmul(
                                            pvs[h][:, :],
                                            vp[:, sc - 1, h * 65:(h + 1) * 65],
                                            at[:, :],
                                            start=(sc - 1 == 0), stop=False)
                                prev = cur
                            for h, at in prev:
                                nc.tensor.matmul(
                                    pvs[h][:, :],
                                    vp[:, 31, h * 65:(h + 1) * 65],
                                    at[:, :], start=False, stop=True)
                        with tc.tile_pool(name="psRR", bufs=1,
                                          space="PSUM") as psRR:
                            for h in range(HC):
                                pb = (h % 2) * 64
                                hp = h // 2
                                pv = pvs[h]
                                rr = rrpool.tile([1, 512], F32, tag="rr")
                                nc.vector.reciprocal(rr, pv[64:65, :])
                                rrb = psRR.tile([64, 512], F32, tag="rrb")
                                nc.tensor.matmul(rrb[:, :], ones64[:, :],
                                                 rr[:, :], start=True, stop=True)
                                rrs = rrpool.tile([64, 512], F32, tag="rrs")
                                nc.vector.tensor_copy(rrs, rrb[:, :])
                                nc.vector.tensor_tensor(
                                    out=ctxT[pb:pb + 64, hp, ksl],
                                    in0=pv[0:64, :], in1=rrs, op=OP.mult)
                    # ---- out-projection for this half + ReduceScatter ----
                    y_sb = ypool.tile([P, 4, D], BF16, tag="y")
                    with tc.tile_pool(name="psF", bufs=2, space="PSUM") as psF:
                        for kt in range(4):
                            kk = slice(half * KH + kt * P,
                                       half * KH + (kt + 1) * P)
                            for db in range(2):
                                ps = psF.tile([P, 512], F32, tag="y")
                                for cc in range(2):
                                    nc.tensor.matmul(
                                        ps[:, :], ctxT[:, cc, kk],
                                        wo_bf[:, cc, db * 512:(db + 1) * 512],
                                        start=(cc == 0), stop=(cc == 1))
                                nc.vector.tensor_copy(
                                    y_sb[:, kt, db * 512:(db + 1) * 512],
                                    ps[:, :])
                    if taps and half == 0:
                        nc.sync.dma_start(out=tap["y"].ap(), in_=y_sb[:, :, :])
                    rs_in = dpool.tile([KH, D], BF16, tag=f"rsin{half}",
                                       name=f"rs_in{half}")
                    nc.sync.dma_start(
                        out=rs_in.rearrange("(t p) d -> p t d", p=P),
                        in_=y_sb[:, :, :])
                    nc.gpsimd.collective_compute(
                        "ReduceScatter", OP.add, replica_groups=groups,
                        ins=[rs_in.opt()], outs=[rs_out[half].ap().opt()])
            if taps:
                nc.sync.dma_start(out=tap["etB"].ap(), in_=etB[:, :, :])
                nc.sync.dma_start(out=tap["ctx"].ap(), in_=ctxT[:, :, :])
            ring_cm.__exit__(None, None, None)
            et_cm.__exit__(None, None, None)

            # ======= residual + LN_f + FFN on my KQ rows (2 chunks) ======
            with (
                tc.tile_pool(name="ffn", bufs=1) as fp,
                tc.tile_pool(name="fstream", bufs=2) as fs,
                tc.tile_pool(name="psH", bufs=4, space="PSUM") as psH,
                tc.tile_pool(name="psF2", bufs=2, space="PSUM") as psF2,
            ):
                w2_bf = fp.tile([P, D2 // P, D], BF16)
                nc.scalar.dma_start(out=w2_bf, in_=w2_d.ap().rearrange(
                    "(c p) n -> p c n", p=P))
                x_sb = fp.tile([P, 2, D], F32)
                for ch in range(2):
                    rs_sb = fs.tile([P, D], BF16, tag="rs")
                    nc.sync.dma_start(out=rs_sb, in_=rs_out[ch].ap())
                    nc.vector.tensor_tensor(out=x_sb[:, ch, :],
                                            in0=qres_sb[:, ch, :],
                                            in1=rs_sb, op=OP.add)
                    nc.vector.tensor_tensor(out=x_sb[:, ch, :],
                                            in0=x_sb[:, ch, :],
                                            in1=bo_bc, op=OP.add)
                    mv, rs = _ln_stats(nc, fs, x_sb[:, ch, :], D)
                    xn = fs.tile([P, D], BF16, tag="xn")
                    nc.vector.tensor_scalar(out=xn, in0=x_sb[:, ch, :],
                                            scalar1=mv[:, 0:1], scalar2=rs,
                                            op0=OP.subtract, op1=OP.mult)
                    xfT = fs.tile([P, DC, P], BF16, tag="xfT")
                    nc.scalar.dma_start_transpose(xfT[:, :, :], xn[:, :])
                    h1n = fs.tile([P, D2], BF16, tag="h1n")
                    for j4 in range(4):
                        sl2 = slice(j4 * 512, (j4 + 1) * 512)
                        ps = psH.tile([P, 512], F32, tag="h1")
                        for dc in range(DC):
                            nc.tensor.matmul(
                                ps[:, :], xfT[:, dc, :], w1_bf[:, dc, sl2],
                                start=(dc == 0), stop=False)
                        nc.tensor.matmul(ps[:, :], ones_row[0:1, 0:P],
                                         b1_bf[0:1, sl2],
                                         start=False, stop=True)
                        nc.scalar.activation(h1n[:, sl2], ps[:, :], AF.Gelu)
                    h1T = fs.tile([P, D2 // P, P], BF16, tag="h1T")
                    nc.scalar.dma_start_transpose(h1T[:, :, :], h1n[:, :])
                    o_sb = fs.tile([P, D], F32, tag="o")
                    for db in range(2):
                        sl = slice(db * 512, (db + 1) * 512)
                        ps = psF2.tile([P, 512], F32, tag="h2")
                        for hc in range(D2 // P):
                            nc.tensor.matmul(
                                ps[:, :], h1T[:, hc, :],
                                w2_bf[:, hc, sl],
                                start=(hc == 0), stop=False)
                        nc.tensor.matmul(ps[:, :], ones_row[0:1, 0:P],
                                         b2_bf[0:1, sl],
                                         start=False, stop=True)
                        nc.vector.tensor_tensor(out=o_sb[:, sl], in0=ps[:, :],
                                                in1=x_sb[:, ch, sl], op=OP.add)
                    nc.sync.dma_start(
                        out=out_d[ch * P:(ch + 1) * P, :], in_=o_sb[:, :])
                if taps:
                    nc.sync.dma_start(out=tap["x"].ap(), in_=x_sb[:, :, :])

    nc.compile()
    return nc


def _get_nc(taps=False):
    key = "nc_taps" if taps else "nc"
    if key not in _CACHE:
        _CACHE[key] = _build_nc(taps=taps)
    return _CACHE[key]


def _softplus(x):
    return float(np.log1p(np.exp(np.float64(x))))


# permuted token order: quarter q's first 128 rows, then quarter q's
# second 128 rows — so half-RS #i delivers each core its chunk i.
_PERM = np.concatenate(
    [np.arange(qq * KQ, qq * KQ + P) for qq in range(4)]
    + [np.arange(qq * KQ + P, (qq + 1) * KQ) for qq in range(4)])


def kernel(**inputs):
    f = lambda name: np.ascontiguousarray(np.asarray(inputs[name], np.float32))
    q = f("q"); kv = f("kv"); ab = f("attn_bias"); ob = f("obs_bias")
    density = f("density")
    c1 = _softplus(inputs["dist_raw"])
    c2 = _softplus(inputs["obs_raw"])
    tg = float(np.tanh(np.float64(np.asarray(inputs["dens_raw"], np.float64))))
    gate = (1.0 + tg * density).astype(np.float32)       # [B, K]

    ln_q_w = f("ln_q_w"); ln_q_b = f("ln_q_b")
    ln_kv_w = f("ln_kv_w"); ln_kv_b = f("ln_kv_b")
    ln_f_w = f("ln_f_w"); ln_f_b = f("ln_f_b")
    scale = np.float32(HD ** -0.5)
    wq = scale * ln_q_w[:, None] * f("wq")
    bq = scale * (ln_q_b @ f("wq") + f("bq"))
    wk = ln_kv_w[:, None] * f("wk"); bk = ln_kv_b @ f("wk") + f("bk")
    wv = ln_kv_w[:, None] * f("wv"); bv = ln_kv_b @ f("wv") + f("bv")
    w1 = ln_f_w[:, None] * f("w1"); b1 = ln_f_b @ f("w1") + f("b1")
    wo = f("wo"); bo = f("bo"); w2 = f("w2"); b2 = f("b2")

    cont = np.ascontiguousarray
    bf = lambda a: np.ascontiguousarray(np.asarray(a, dtype=ml_dtypes.bfloat16))
    perm = _PERM
    in_maps = []
    per_batch = []
    for b in range(B):
        bt = (c1 * ab[b] + c2 * ob[b]).T[:, perm]   # [S, K] permuted cols
        gp = gate[b][perm]
        per_batch.append({
            "q": bf(q[b][perm]), "kv": bf(kv[b]),
            "bt": cont(bt),
            "gate": cont(gp[:, None]),
            "gbc": cont(np.broadcast_to(gp[None, :], (P, K))),
            "growb": bf(gp[None, :]),
        })
    for c in range(N_CORES):
        b, hq = divmod(c, 4)
        hs = slice(hq * DH, (hq + 1) * DH)
        ks = slice(hq * KQ, (hq + 1) * KQ)
        m = dict(per_batch[b])
        m.update({
            "q_res": cont(q[b, ks]),
            "wq": bf(wq[:, hs]), "wk": bf(wk[:, hs]), "wv": bf(wv[:, hs]),
            "bq": bf(bq[None, hs]), "bk": bf(bk[None, hs]),
            "bv": bf(bv[None, hs]),
            "wo": bf(wo[hs, :]), "bo": cont(bo[None, :]),
            "w1": bf(w1), "b1": bf(b1[None, :]),
            "w2": bf(w2), "b2": bf(b2[None, :]),
        })
        in_maps.append(m)

    global _last_in_maps
    _last_in_maps = in_maps
    nc = _get_nc()
    res = bass_utils.run_bass_kernel_spmd(
        nc, in_maps, core_ids=list(range(N_CORES)))
    out = np.empty((B, K, D), np.float32)
    for c in range(N_CORES):
        b, hq = divmod(c, 4)
        out[b, hq * KQ:(hq + 1) * KQ, :] = res.results[c]["xq"]
    return out
